# revision 6
# baseline (speedup 1.0000x reference)
"""Trainium2 Bass kernel for nn_MixedLinear_89979564851799 — hybrid fp8/bf16.

y = x @ w_t (8192x4096x4096), data-parallel over tokens across 8 cores.

Per-core (M=1024, K=4096, N=4096), the contraction is split:
  - 10 k-blocks (the exactly-fp8-representable fp8 partition cols
    3584:4096 plus fp4-partition cols 0:768) run as 5 DoubleRow fp8
    matmuls per output tile: lhsT [128,2,128] x rhs [128,2,512] ->
    [128,512], K=256 per instruction at 2x bf16 FLOP rate (measured
    224ns vs 221ns for half the work).
  - 22 k-blocks (cols 768:3584) run as bf16 matmuls.
Scales: x8 = e4m3(x*16), w8 = e4m3(w*1024); bf16 W is pre-scaled by 2^14
so all PSUM contributions share scale 2^14; the PSUM->SBUF copy is a
tensor_scalar_mul by 2^-14.  Measured rel err vs the fp32 reference:
1.879e-2 (gate 2e-2); the all-bf16 baseline was 2.26e-3 at 470us, this
kernel ~395us at full clock.
"""

import os
import numpy as np
import ml_dtypes

P = 128
TOKENS, IN, OUT = 8192, 4096, 4096
NCORES = 8
M_PER_CORE = TOKENS // NCORES      # 1024
MT = M_PER_CORE // P               # 8 m-tiles
NCH = 8                            # n chunks
NW = OUT // NCH                    # 512
NS = 5                             # fp8 k256 steps
KT16 = 22                          # bf16 k-tiles
KG = 2                             # bf16 k-groups per n-chunk load
KTG = KT16 // KG                   # 11 k-tiles per group
SX = 16.0                          # fp8 x scale
SW = 1024.0                        # fp8 w scale
OSCALE = 2.0 ** -14                # output copy scale (1/(SX*SW))

LAST_RESULT = None
_BUILT = None


def _patch_tile_drain():
    """The walrus build in this container rejects instructions carrying more
    than one sync-wait (CoreV3GenImpl setupSyncWait: "Too many sync wait
    commands").  Tile's scheduler freely assigns several waits to one
    instruction, so (a) wrap _commit_instruction to hoist extra waits onto
    single-wait NOPs on the same engine just before the offender, and
    (b) split the kernel-tail Drain (which collects one wait per DMA queue)
    into a chain of single-wait Drains."""
    import concourse.tile as tile_mod
    import concourse.mybir as mybir
    import bass_rust
    from concourse.vector_clock import ScopedClock

    if getattr(tile_mod.TileContext, "_single_wait_drain_patch", False):
        return

    orig_commit = tile_mod.TileContext._commit_instruction

    def _commit_instruction(self, inst, lazy_reg_writes=True):
        si = getattr(inst, "sync_info", None)
        if (
            si is not None
            and len(si.on_wait) > 1
            and inst.engine != mybir.EngineType.Unassigned
        ):
            waits = list(si.on_wait)
            for w in waits[:-1]:
                nop = mybir.InstNoOp(
                    name=self.nc.get_next_instruction_name(),
                    engine=inst.engine,
                    sync_info=mybir.SyncInfo(on_wait=[w], on_update=[]),
                    bass_nofuse=True,
                )
                orig_commit(self, nop, lazy_reg_writes=False)
            inst.sync_info = mybir.SyncInfo(
                on_wait=[waits[-1]], on_update=list(si.on_update)
            )
        return orig_commit(self, inst, lazy_reg_writes)

    tile_mod.TileContext._commit_instruction = _commit_instruction

    def _drain_and_barrier(self, tick_clock, wait_clock):
        drain_inst = self.nc.sync.drain()
        wait_clock.add_sem_waits(
            drain_inst.ins, ScopedClock({None: tick_clock.global_clock})
        )
        si = drain_inst.ins.sync_info
        if si is not None and len(si.on_wait) > 1:
            waits = list(si.on_wait)
            drain_inst.ins.sync_info = bass_rust.SyncInfo(
                on_wait=[waits[0]], on_update=list(si.on_update)
            )
            engines = [self.nc.scalar, self.nc.vector, self.nc.gpsimd,
                       self.nc.tensor, self.nc.sync]
            for k, w in enumerate(waits[1:]):
                extra = engines[k % len(engines)].drain()
                extra.ins.sync_info = bass_rust.SyncInfo(on_wait=[w], on_update=[])
        self.nc.all_engine_barrier()
        popped = self.nc._tile_sem_poison_stack.pop()
        assert popped is self._sem_poison
        self.nc.clear_and_free_semaphores(list(self.sems.allocated().values()))
        self.nc.all_engine_barrier()

    tile_mod.TileContext._drain_and_barrier = _drain_and_barrier
    tile_mod.TileContext._single_wait_drain_patch = True


def _build():
    global _BUILT
    if _BUILT is not None:
        return _BUILT
    import concourse.bass as bass
    import concourse.tile as tile
    from concourse import mybir

    _patch_tile_drain()
    DR = mybir.MatmulPerfMode.DoubleRow

    nc = bass.Bass("TRN2", debug=False)
    # fp8 operands: x8 [s, p, msplit, j, m256]; w8 [nch, s, p, j, n]
    x8_d = nc.dram_tensor(
        "x8", [NS, P, 4, 2, M_PER_CORE // 4], mybir.dt.float8e4, kind="ExternalInput"
    ).ap()
    w8_d = nc.dram_tensor(
        "w8", [NCH, NS, P, 2, NW], mybir.dt.float8e4, kind="ExternalInput"
    ).ap()
    # bf16 operands: xb [mt, p, kg, ktg, m]; wb [nch, kg, p, ktg, n]
    xb_d = nc.dram_tensor(
        "xb", [MT, P, KG, KTG, P], mybir.dt.bfloat16, kind="ExternalInput"
    ).ap()
    wb_d = nc.dram_tensor(
        "wb", [NCH, KG, P, KTG, NW], mybir.dt.bfloat16, kind="ExternalInput"
    ).ap()
    y_d = nc.dram_tensor(
        "y", [M_PER_CORE, OUT], mybir.dt.float32, kind="ExternalOutput"
    ).ap()

    with tile.TileContext(nc) as tc:
        with (
            tc.tile_pool(name="x8p", bufs=1) as x8_pool,
            tc.tile_pool(name="xbp", bufs=1) as xb_pool,
            tc.tile_pool(name="w8p", bufs=3) as w8_pool,
            tc.tile_pool(name="wbp", bufs=4) as wb_pool,
            tc.tile_pool(name="y", bufs=8) as y_pool,
            tc.tile_pool(name="ps", bufs=8, space="PSUM") as ps_pool,
        ):
            # --- PE p-state warmup: the tensor engine reaches max clock only
            # after ~3us of continuous execution; burn dummy matmuls on a
            # memset tile while the first DMAs land so the real stream starts
            # at full speed.  The dummy PSUM writes land in a pool buffer
            # that is reset (start=True) when the pool rotates back to it.
            warm = xb_pool.tile([P, NW], mybir.dt.bfloat16, name="warm",
                                tag="warm")
            nc.vector.memset(warm[:], 0.03125)
            psw = ps_pool.tile([16, NW], mybir.dt.float32, name="psw", tag="ps")
            for _ in range(8):
                nc.tensor.matmul(psw[:], lhsT=warm[:, :16], rhs=warm[:],
                                 start=True, stop=True)

            # --- resident x tiles ---
            x8_sbs = [None] * NS

            def load_x8(s, split, eng=None):
                eng = eng or nc.sync
                t = x8_pool.tile([P, 4, 2, M_PER_CORE // 4], mybir.dt.float8e4,
                                 name=f"x8_{s}", tag=f"x8_{s}")
                step = 4 // split
                for u in range(split):
                    eng.dma_start(
                        t[:, u * step:(u + 1) * step],
                        x8_d[s][:, u * step:(u + 1) * step],
                    )
                x8_sbs[s] = t

            def x8_lhsT(s, mt):
                return x8_sbs[s][:, mt // 2, :, (mt % 2) * P:(mt % 2 + 1) * P]

            xb_sbs = [None] * MT

            def load_xb(mt, kgs=None, eng=None):
                eng = eng or nc.sync
                if xb_sbs[mt] is None:
                    xb_sbs[mt] = xb_pool.tile(
                        [P, KG, KTG, P], mybir.dt.bfloat16,
                        name=f"xb_{mt}", tag=f"xb_{mt}"
                    )
                for kg in (kgs if kgs is not None else range(KG)):
                    eng.dma_start(xb_sbs[mt][:, kg], xb_d[mt, :, kg])

            def load_w8_s(nch, s, split=1):
                t = w8_pool.tile([P, 2, NW], mybir.dt.float8e4,
                                 name=f"w8_{nch}_{s}", tag=f"w8s{s}")
                if split > 1:
                    w = NW // split
                    for u in range(split):
                        nc.sync.dma_start(
                            t[:, :, u * w:(u + 1) * w],
                            w8_d[nch, s, :, :, u * w:(u + 1) * w],
                        )
                else:
                    nc.sync.dma_start(t[:], w8_d[nch, s])
                return t

            def load_w8(nch, split=1):
                return [load_w8_s(nch, s, split if s == 0 else 1)
                        for s in range(NS)]

            def load_wb_kg(nch, kg, split=2):
                t = wb_pool.tile([P, KTG, NW], mybir.dt.bfloat16,
                                 name=f"wb_{nch}_{kg}", tag=f"wb{kg}")
                bounds = [round(KTG * u / split) for u in range(split + 1)]
                for u in range(split):
                    nc.sync.dma_start(
                        t[:, bounds[u]:bounds[u + 1]],
                        wb_d[nch, kg, :, bounds[u]:bounds[u + 1]],
                    )
                return t

            # --- head: issue order matched to PE need-time.  DMA issue costs
            # ~600ns of engine time and Tile serializes transfers through a
            # shared pool of 8 semaphores, so use few, well-sized pieces
            # (contiguous elems) ordered exactly by first use.
            # sync family: w8[0] + wb[0]; scalar family: x8 + xb.
            w8_t = [load_w8_s(0, 0)]
            load_x8(0, split=2, eng=nc.scalar)
            w8_t.append(load_w8_s(0, 1))
            load_x8(1, split=2, eng=nc.scalar)
            wb0_kg0 = wb_pool.tile([P, KTG, NW], mybir.dt.bfloat16,
                                   name="wb_0_0", tag="wb0")
            nc.sync.dma_start(wb0_kg0[:, 0:3], wb_d[0, 0, :, 0:3])
            w8_t.append(load_w8_s(0, 2))
            load_x8(2, split=1, eng=nc.scalar)
            nc.sync.dma_start(wb0_kg0[:, 3:7], wb_d[0, 0, :, 3:7])
            for s in range(3, NS):
                w8_t.append(load_w8_s(0, s))
                load_x8(s, split=1, eng=nc.scalar)
            nc.sync.dma_start(wb0_kg0[:, 7:11], wb_d[0, 0, :, 7:11])
            wb_t = [wb0_kg0]
            for mt in range(0, 4):
                load_xb(mt, kgs=[0], eng=nc.scalar)
            for mt in range(4, MT):
                load_xb(mt, kgs=[0], eng=nc.sync)
            wb_t.append(load_wb_kg(0, 1, split=3))
            for mt in range(0, 4):
                load_xb(mt, kgs=[1], eng=nc.scalar)
            for mt in range(4, MT):
                load_xb(mt, kgs=[1], eng=nc.scalar)

            def emit_out(nch, mt, y_sb, last):
                nsplit = 4 if last else 2
                w = NW // nsplit
                for u in range(nsplit):
                    eng = nc.scalar if (not last or u % 2 == 0) else nc.sync
                    eng.dma_start(
                        y_d[
                            mt * P:(mt + 1) * P,
                            nch * NW + u * w: nch * NW + (u + 1) * w,
                        ],
                        y_sb[:, u * w:(u + 1) * w],
                    )

            def fp8_mm(ps, s, mt, w8_t, start):
                nc.tensor.matmul(
                    ps[:],
                    lhsT=x8_lhsT(s, mt),
                    rhs=w8_t[s][:],
                    start=start,
                    stop=False,
                    perf_mode=DR,
                )

            def bf16_mm(ps, kt, mt, wb_t, stop, start=False):
                nc.tensor.matmul(
                    ps[:],
                    lhsT=xb_sbs[mt][:, kt // KTG, kt % KTG, :],
                    rhs=wb_t[kt // KTG][:, kt % KTG, :],
                    start=start,
                    stop=stop,
                )

            # --- chunk 0: s-outer fp8 then kt-outer bf16, so the PE's data
            # needs trail the DMA arrival order (w8s0+x8s0 -> w8s*/x8s* ->
            # wb kg0 -> kg1 -> kg2) while the first loads stream in.
            ps_t = [ps_pool.tile([P, NW], mybir.dt.float32, name="ps")
                    for _ in range(MT)]
            for s in range(NS):
                for mt in range(MT):
                    fp8_mm(ps_t[mt], s, mt, w8_t, start=(s == 0))
            for kt in range(KT16 - 1):
                for mt in range(MT):
                    bf16_mm(ps_t[mt], kt, mt, wb_t, stop=False)
            for mt in range(MT):
                bf16_mm(ps_t[mt], KT16 - 1, mt, wb_t, stop=True)
                y_sb = y_pool.tile([P, NW], mybir.dt.float32, name="y_sb")
                nc.vector.tensor_scalar_mul(y_sb[:], ps_t[mt][:], OSCALE)
                emit_out(0, mt, y_sb, last=False)

            # --- chunks 1..7: per-(nch,mt) groups ---
            for nch in range(1, NCH):
                w8_t = load_w8(nch)
                wb_t = [load_wb_kg(nch, kg) for kg in range(KG)]
                for mt in range(MT):
                    ps = ps_pool.tile([P, NW], mybir.dt.float32, name="ps")
                    for s in range(NS):
                        fp8_mm(ps, s, mt, w8_t, start=(s == 0))
                    for kt in range(KT16):
                        bf16_mm(ps, kt, mt, wb_t, stop=(kt == KT16 - 1))
                    y_sb = y_pool.tile([P, NW], mybir.dt.float32, name="y_sb")
                    nc.vector.tensor_scalar_mul(y_sb[:], ps[:], OSCALE)
                    emit_out(nch, mt, y_sb,
                             last=(nch == NCH - 1 and mt == MT - 1))
    _BUILT = nc
    return nc


def _q8(a, scale):
    v = np.clip(a * np.float32(scale), -240.0, 240.0)
    return np.asarray(v, dtype=ml_dtypes.float8_e4m3)


def prepare(inputs):
    """Host-side prep: quantize/cast/tile inputs per core; return
    (nc, in_maps, assemble)."""
    x = np.asarray(inputs["x"], dtype=np.float32)
    w_t = np.asarray(inputs["w_t"], dtype=np.float32)

    nc = _build()

    cols8 = np.concatenate([np.arange(3584, 4096), np.arange(0, 768)])
    cols16 = np.arange(768, 3584)

    xt = np.ascontiguousarray(x.T)           # [IN, TOKENS] fp32
    # fp8 weights: [nch, s, p, j, n]
    w8f = w_t[cols8]                          # [1024, OUT]
    w8q = _q8(w8f, SW)                        # e4m3
    w8_tiled = np.ascontiguousarray(
        w8q.reshape(NS, 2, P, NCH, NW).transpose(3, 0, 2, 1, 4)
    )
    # bf16 weights, pre-scaled by 2^14: [nch, kg, p, ktg, n]
    wbf = (w_t[cols16] * np.float32(SX * SW)).astype(ml_dtypes.bfloat16)
    wb_tiled = np.ascontiguousarray(
        wbf.reshape(KG, KTG, P, NCH, NW).transpose(3, 0, 2, 1, 4)
    )

    x8_all = _q8(xt[cols8], SX)               # [1024, TOKENS]
    xb_all = xt[cols16].astype(ml_dtypes.bfloat16)  # [3072, TOKENS]

    in_maps = []
    for c in range(NCORES):
        sl = slice(c * M_PER_CORE, (c + 1) * M_PER_CORE)
        # x8: [s, p, msplit, j, m256]
        x8c = np.ascontiguousarray(
            x8_all[:, sl].reshape(NS, 2, P, 4, M_PER_CORE // 4)
            .transpose(0, 2, 3, 1, 4)
        )
        # xb: [mt, p, kg, ktg, m]
        xbc = np.ascontiguousarray(
            xb_all[:, sl].reshape(KG, KTG, P, MT, P).transpose(3, 2, 0, 1, 4)
        )
        in_maps.append({"x8": x8c, "xb": xbc, "w8": w8_tiled, "wb": wb_tiled})

    def assemble(outs_by_name, n_cores):
        y = outs_by_name["y"].reshape(n_cores, M_PER_CORE, OUT)
        return np.concatenate([y[i] for i in range(n_cores)], axis=0)

    return nc, in_maps, assemble


def kernel(x, w_q_fp4, w_os_fp4, w_is_fp4, w_t, w_q_fp8, w_s_fp8):
    global LAST_RESULT
    from concourse.bass_utils import run_bass_kernel_spmd

    nc, in_maps, assemble = prepare({"x": x, "w_t": w_t})
    res = None
    for attempt in range(3):
        try:
            res = run_bass_kernel_spmd(
                nc,
                in_maps,
                list(range(NCORES)),
                trace=bool(os.environ.get("BASS_TRACE")),
            )
            break
        except Exception:
            # transient device errors (e.g. NRT_EXEC_UNIT_UNRECOVERABLE)
            # have been observed once and succeeded on retry
            if attempt == 2:
                raise
    LAST_RESULT = res
    return assemble({"y": np.stack([res.results[i]["y"] for i in range(NCORES)])},
                    NCORES)
